# revision 39
# baseline (speedup 1.0000x reference)
"""Involution kernel for Trainium2, 8-core data-parallel (1 batch image per core).

Reference computation (per image, NHWC, C=64, G=4 groups, K=3, reduction 4):
    t    = relu(BN(x @ w1 + b1))            # [H,W,16]
    kern = t @ w2 + b2                      # [H,W,36], e = (ki*3+kj)*4 + g
    out[h,w,c] = sum_p kern[h,w, 4p + c%4] * xpad[h+di, w+dj, c]

Device strategy (per core = one image):
  * 256 subtiles of 12x12 interior; subtile on the partition axis (2 blocks
    of 128).  x2 holds 14x14 1-ring windows so the 3x3 taps are free-dim
    offsets.
  * matmul1 processes interior pixels in (block, q, subtile) order; outputs
    are stacked 6x16 rows deep in PSUM and relu-evacuated in one Scalar
    (ACT) instruction per 3072 pixels.
  * matmul2 contracts t' (16) against w2 per q-position; b2 is added with a
    rank-1 ones x b2 matmul per PSUM round; Scalar engine evacuates kern.
  * Involution: 9 per-tap elementwise products (DVE x7 + GPSIMD x2, all
    bf16 step-1 so DVE runs 2x mode), then the 9-way add tree runs on the
    PE as identity-stationary matmuls accumulating in PSUM (fp32), with one
    product pair pre-folded on DVE.  Scalar engine evacuates acc to bf16.
  * All bulk data bf16; host folds BN into w1 and builds the two layouts.
"""

import os
import numpy as np
import ml_dtypes

import concourse.bass as bass
import concourse.bacc as bacc
import concourse.mybir as mybir
from concourse.tile import TileContext
from concourse.bass_utils import run_bass_kernel_spmd

BF16 = mybir.dt.bfloat16
FP32 = mybir.dt.float32
NPF32 = np.float32
NPBF16 = ml_dtypes.bfloat16
AF = mybir.ActivationFunctionType

B, H, W, C = 8, 192, 192, 64
G, K, CR, E = 4, 3, 16, 36
BN_EPS = 1e-3
S = 12                  # subtile interior
S1 = S + 2              # 1-ring padded subtile
NG = H // S             # 16 subtiles per axis
NST = NG * NG           # 256 subtiles
NB = 2                  # partition blocks of 128 subtiles
NQ = S * S              # 144 interior positions per subtile
NPIX = NST * NQ         # 36864 interior pixels
NPB = NPIX // NB        # 18432 pixels per block
F2 = S1 * S1 * C        # 12544 x2 free elems per subtile
FO = NQ * C             # 9216 out free elems per subtile
RND = 2048              # mm1 pixels per round (4 groups x 512)
NRND = NPIX // RND      # 18 rounds (9 per block)
QPP = 28                # q positions per mm2 psum round
CHT = 1536              # product tile free size (2 qh rows x 12 qw x 64)
POOL_TAPS = (5, 6)      # taps computed on GPSIMD
LAST_TAP = 8            # tap merged during the DVE psum-readback add

_CACHE = {}


def _build_program():
    if "nc" in _CACHE:
        return _CACHE["nc"]
    stage = os.environ.get("KSTAGE", "full")
    nc = bacc.Bacc(None, target_bir_lowering=False)
    x2_d = nc.dram_tensor("x2", [NST, F2], BF16, kind="ExternalInput")
    xt_d = nc.dram_tensor("xt", [C + 1, NPIX], BF16, kind="ExternalInput")
    w1_d = nc.dram_tensor("w1a", [C + 1, 32], BF16, kind="ExternalInput")
    w2_d = nc.dram_tensor("w2b", [CR + 1, E], BF16, kind="ExternalInput")
    id_d = nc.dram_tensor("ident", [128, 128], BF16, kind="ExternalInput")
    o_d = nc.dram_tensor("o", [NST, FO], BF16, kind="ExternalOutput")

    with TileContext(nc) as tc:
        with (
            tc.tile_pool(name="const", bufs=1) as cpool,
            tc.tile_pool(name="x2p", bufs=2) as x2pool,
            tc.tile_pool(name="xtp", bufs=4) as xtpool,
            tc.tile_pool(name="tpp", bufs=1) as tp0pool,
            tc.tile_pool(name="tpsr", bufs=2) as tpspool,
            tc.tile_pool(name="kernp", bufs=2) as kpool,
            tc.tile_pool(name="oevp", bufs=2) as oevpool,
            tc.tile_pool(name="prodp", bufs=2) as ppool,
            tc.tile_pool(name="ps1", bufs=1, space="PSUM") as ps1pool,
            tc.tile_pool(name="ps2", bufs=1, space="PSUM") as ps2pool,
            tc.tile_pool(name="psa", bufs=4, space="PSUM") as psapool,
        ):
            w1t = cpool.tile([C + 1, 32], BF16, tag="w1")
            w2t = cpool.tile([CR + 1, E], BF16, tag="w2")
            idt = cpool.tile([128, 128], BF16, tag="id")
            nc.sync.dma_start(w1t[:], w1_d[:])
            nc.sync.dma_start(w2t[:], w2_d[:])

            # mm1 writes 4 stacked 32-row bands (PE col tiling); the DVE
            # relu evacuates psum to a staging ring, then DMA relocates each
            # band's 17 rows (16 d + bias row) to partition-base-0 tp0 so
            # mm2 can run untiled (mixing PE row- and col-tiling in one
            # program crashes the runtime).  tp0 col = block-local pixel.
            tp0 = tp0pool.tile([CR + 1, NPB], BF16, tag="tp0")

            xt_pref = {}

            def xt_prefetch(r):
                if r in xt_pref:
                    return
                t = xtpool.tile([C + 1, RND], BF16, tag="xt")
                nc.sync.dma_start(t[:], xt_d[:, r * RND:(r + 1) * RND])
                xt_pref[r] = t

            x2_backlog = {}

            def mm1_round2(r, nhalves=2):
                # two xt rounds (4096 pixels) into one [128,1024] psum tile,
                # evacuated with a single DVE relu (ACT engine is unusable in
                # this runtime, so all PSUM readback goes through DVE at 1x)
                ps1 = ps1pool.tile([128, 1024], FP32, tag="ps1")
                tps = tpspool.tile([128, 1024], BF16, tag="tps")
                for half in range(nhalves):
                    rr = r + half
                    if rr not in xt_pref:
                        xt_prefetch(rr)
                    xtt = xt_pref.pop(rr)
                    for nxt in (rr + 2, rr + 3):
                        if nxt < NRND and nxt // 9 == rr // 9:
                            xt_prefetch(nxt)
                    for x2t, b, lo, hi in x2_backlog.pop(rr, ()):
                        nc.sync.dma_start(
                            x2t[:, lo * S1 * C:hi * S1 * C],
                            x2_d[b * 128:(b + 1) * 128,
                                 lo * S1 * C:hi * S1 * C])
                    for g in range(4):
                        nc.tensor.matmul(
                            ps1[32 * g:32 * g + 32,
                                half * 512:half * 512 + 512],
                            w1t[:],
                            xtt[:, g * 512:(g + 1) * 512],
                            start=True, stop=True,
                            tile_position=(0, 32 * g))
                nc.vector.tensor_scalar_max(
                    tps[:, 0:nhalves * 512], ps1[:, 0:nhalves * 512], 0.0)
                tp0v = tp0[:].rearrange("p (rr c) -> p rr c", c=RND)
                tpsv = tps[:].rearrange("p (hh c) -> p hh c", c=512)
                for g in range(4):
                    nc.sync.dma_start(
                        tp0v[:, r % 9:r % 9 + nhalves,
                             g * 512:g * 512 + 512],
                        tpsv[32 * g:32 * g + CR + 1, 0:nhalves, :])

            def mm2_round(b, q0, nq, kernt):
                # 14 q per 512-fp32 psum bank (36-wide outputs cannot cross
                # a bank boundary); evac reads both banks with a strided AP
                ps2 = ps2pool.tile([128, 1024], FP32, tag="ps2")
                for qi in range(nq):
                    ploc = (q0 + qi) * 128
                    off = (qi // 14) * 512 + (qi % 14) * E
                    nc.tensor.matmul(
                        ps2[:, off:off + E],
                        tp0[:, ploc:ploc + 128],
                        w2t[:],
                        start=True, stop=True)
                for bk in range((nq + 13) // 14):
                    qlo, qhi = bk * 14, min(nq, bk * 14 + 14)
                    nc.vector.tensor_copy(
                        kernt[:, (q0 + qlo) * E:(q0 + qhi) * E],
                        ps2[:, bk * 512:bk * 512 + (qhi - qlo) * E])

            def involution_row(b, a, x2t, kernt):
                # chunk-row a: out rows qh in [2a, 2a+2), all 12 qw, 64 ch
                xv = x2t[:].rearrange("p (h wc) -> p h wc", h=S1)
                kv = kernt[:].rearrange("p (qh qw e) -> p qh qw e", qh=S, e=E)
                prods = [None] * 9
                for p in list(POOL_TAPS) + [p for p in range(9)
                                            if p not in POOL_TAPS]:
                    di, dj = p // 3, p % 3
                    xop = xv[:, 2 * a + di:2 * a + di + 2,
                             dj * C:dj * C + S * C]
                    krep = kv[:, 2 * a:2 * a + 2, :, 4 * p:4 * p + 4] \
                        .unsqueeze(3).broadcast_to([128, 2, S, CR, 4])
                    pt = ppool.tile([128, CHT], BF16, tag=f"prod{p}")
                    eng = nc.gpsimd if p in POOL_TAPS else nc.vector
                    eng.tensor_tensor(pt[:], xop, krep, mybir.AluOpType.mult)
                    prods[p] = pt
                srcs = [prods[p] for p in range(9)
                        if p != LAST_TAP and p not in POOL_TAPS]
                srcs += [prods[p] for p in POOL_TAPS]
                psas = []
                for s in range(CHT // 512):
                    psa = psapool.tile([128, 512], FP32, tag="psa")
                    for j, src in enumerate(srcs):
                        nc.tensor.matmul(
                            psa[:], idt[:], src[:, s * 512:(s + 1) * 512],
                            start=(j == 0), stop=(j == len(srcs) - 1))
                    psas.append(psa)
                return psas, prods[LAST_TAP]

            def evac_row(b, a, psas, plast, split_dma):
                # fused psum readback + last tap add (DVE, 1x from PSUM)
                oev = oevpool.tile([128, CHT], BF16, tag="oev")
                for s, psa in enumerate(psas):
                    nc.vector.tensor_tensor(
                        oev[:, s * 512:(s + 1) * 512], psa[:],
                        plast[:, s * 512:(s + 1) * 512], mybir.AluOpType.add)
                    if split_dma:
                        nc.sync.dma_start(
                            o_d[b * 128:(b + 1) * 128,
                                a * CHT + s * 512:a * CHT + (s + 1) * 512],
                            oev[:, s * 512:(s + 1) * 512])
                if not split_dma:
                    nc.sync.dma_start(
                        o_d[b * 128:(b + 1) * 128, a * CHT:(a + 1) * CHT],
                        oev[:])

            # ---- fully pipelined schedule ----
            # involution row a of a block needs kern q < (2a+2)*12 -> mm2
            # rounds through q0 = ceil-cover; mm2(q0) needs the mm1 round
            # covering pixel (q0+nq)*128-1 (PE executes in emission order,
            # so every producer must be emitted before its consumer).
            def block_schedule(b, x2t, kernt, la0=2):
                xt_prefetch(b * 9)
                xt_prefetch(b * 9 + 1)
                rloc = list(range(9))        # mm1 rounds of this block
                q1 = list(range(0, NQ, QPP))  # mm2 psum rounds

                def rr_needed(q0):
                    nq = min(QPP, NQ - q0)
                    return ((q0 + nq) * 128 - 1) // RND

                for a in range(6):
                    # run the front-end `la` rows ahead of the involution so
                    # kern evacs have slack over DMA queueing jitter
                    la = la0 if a == 0 else 2
                    qmax = (2 * (a + la) + 2) * S
                    while q1 and q1[0] < qmax:
                        q0 = q1[0]
                        while rloc and rloc[0] <= rr_needed(q0):
                            r0 = rloc.pop(0)
                            nh = 1
                            if rloc and rloc[0] == r0 + 1:
                                rloc.pop(0)
                                nh = 2
                            mm1_round2(b * 9 + r0, nh)
                        q1.pop(0)
                        mm2_round(b, q0, min(QPP, NQ - q0), kernt)
                    yield a

            x2t0 = x2pool.tile([128, F2], BF16, tag="x2")
            x2t1 = x2pool.tile([128, F2], BF16, tag="x2")
            kern0 = kpool.tile([128, NQ * E], BF16, tag="kern")
            kern1 = kpool.tile([128, NQ * E], BF16, tag="kern")

            xt_prefetch(0)
            xt_prefetch(1)
            nc.sync.dma_start(idt[:], id_d[:])
            nc.sync.dma_start(x2t0[:, 0:4 * S1 * C], x2_d[0:128, 0:4 * S1 * C])
            x2_backlog.update({
                1: [(x2t0, 0, 4, 8)],
                3: [(x2t0, 0, 8, S1)],
                6: [(x2t1, 1, 0, 4)],
                7: [(x2t1, 1, 4, 8)],
                8: [(x2t1, 1, 8, S1)],
            })
            # interleave the block boundary: b1 row 0 runs between b0 rows
            # 4 and 5 so DVE keeps streaming while b1's front-end drains
            gens = {0: block_schedule(0, x2t0, kern0),
                    1: block_schedule(1, x2t1, kern1, la0=0)}
            ready = {0: -1, 1: -1}
            ctx = {0: (x2t0, kern0), 1: (x2t1, kern1)}
            order = [(0, 0), (0, 1), (0, 2), (0, 3), (0, 4), (1, 0),
                     (0, 5), (1, 1), (1, 2), (1, 3), (1, 4), (1, 5)]
            if stage == "mm1":
                for r in range(0, NRND, 2):
                    mm1_round2(r)
                nc.sync.dma_start(o_d[0:128, 0:NRND * 512], tpst[:])
            elif stage == "fe":
                for b, kt in ((0, kern0), (1, kern1)):
                    for r in range(9 * b, 9 * b + 9, 2):
                        mm1_round2(r, 2 if r % 9 < 8 else 1)
                    for q0 in range(0, NQ, QPP):
                        mm2_round(b, q0, min(QPP, NQ - q0), kt)
                    nc.sync.dma_start(
                        o_d[b * 128:(b + 1) * 128, 0:NQ * E], kt[:])
            elif stage == "inv1":
                for r in range(0, NRND, 2):
                    mm1_round2(r)
                for b, kt in ((0, kern0), (1, kern1)):
                    for q0 in range(0, NQ, QPP):
                        mm2_round(b, q0, min(QPP, NQ - q0), kt)
                pend = None
                for b, a in order:
                    psas, plast = involution_row(b, a, *ctx[b])
                    if pend is not None:
                        evac_row(*pend, split_dma=False)
                    pend = (b, a, psas, plast)
                evac_row(*pend, split_dma=True)
            else:
                pending = None
                for b, a in order:
                    while ready[b] < a:
                        ready[b] = next(gens[b])
                    psas, plast = involution_row(b, a, *ctx[b])
                    if pending is not None:
                        evac_row(*pending, split_dma=False)
                    pending = (b, a, psas, plast)
                    if (b, a) == (0, 2):
                        ready[1] = next(gens[1])  # b1 row-0 front, minimal
                evac_row(*pending, split_dma=True)
    nc.compile()
    _CACHE["nc"] = nc
    return nc


def _host_prep(x, w1, b1, gamma, beta, mean, var, w2, b2):
    """Per-core input maps. x: [8,192,192,64] f32."""
    a = (gamma / np.sqrt(var + BN_EPS)).astype(NPF32)
    w1a = np.zeros((C + 1, 32), dtype=NPF32)
    w1a[:C, :CR] = w1 * a[None, :]
    w1a[C, :CR] = b1 * a + (beta - mean * a)
    w1a[C, CR] = 1.0  # bias column -> tp pad row 16 of each band is 1.0
    w1a = w1a.astype(NPBF16)
    w2b = np.zeros((CR + 1, E), dtype=NPF32)
    w2b[:CR] = w2
    w2b[CR] = b2
    w2b = w2b.astype(NPBF16)
    ident = np.eye(128, dtype=NPF32).astype(NPBF16)

    xb = x.astype(NPBF16)
    in_maps = []
    for bi in range(B):
        xi = xb[bi]
        xp1 = np.zeros((H + 2, W + 2, C), dtype=NPBF16)
        xp1[1:-1, 1:-1] = xi
        st = xp1.strides
        win1 = np.lib.stride_tricks.as_strided(
            xp1, (NG, NG, S1, S1, C), (st[0] * S, st[1] * S, st[0], st[1], st[2]))
        x2 = np.ascontiguousarray(win1).reshape(NST, F2)
        # xt: interior pixels in (block, q, subtile-within-block) order
        arr = xi.reshape(2, 8, S, NG, S, C).transpose(0, 2, 4, 1, 3, 5)
        xt = np.empty((C + 1, NPIX), dtype=NPBF16)
        xt[:C] = arr.reshape(NPIX, C).T
        xt[C] = NPBF16(1.0)
        in_maps.append({"x2": x2, "xt": xt, "w1a": w1a, "w2b": w2b,
                        "ident": ident})
    return in_maps


def kernel(x, w1, b1, gamma, beta, mean, var, w2, b2, _bench=None):
    nc = _build_program()
    in_maps = _host_prep(np.asarray(x), np.asarray(w1), np.asarray(b1),
                         np.asarray(gamma), np.asarray(beta), np.asarray(mean),
                         np.asarray(var), np.asarray(w2), np.asarray(b2))
    kw = dict(_bench) if _bench else {}
    res = run_bass_kernel_spmd(nc, in_maps, core_ids=list(range(B)), **kw)
    if _bench is not None:
        _bench["result"] = res
    out = np.empty((B, H, W, C), dtype=NPF32)
    for bi in range(B):
        ob = res.results[bi]["o"].reshape(NG, NG, 6, 2, S, C).astype(NPF32)
        out[bi] = ob.transpose(0, 2, 3, 1, 4, 5).reshape(H, W, C)
    return out
